# revision 1
# baseline (speedup 1.0000x reference)
"""Trainium2 Bass kernel for nn_CRFLoss (single-path CRF numerator loss).

Math (matches the reference):
  loss = ( sum_b [ emis_b + lm_b ] ) / num_tokens
  emis_b = sum over valid positions p of log_probs[b, p, labels[b,p]]
  lm_b   = start_lp[s0] + sum_t trans[s_{t-1}, s_t] + fin[s_{T-1}]
           over the sequence of valid labels (s = label - 1)
  where start_lp = log_softmax(A[:L]),
        rows     = log_softmax(A[L:].reshape(L, L+1)), trans = rows[:, :L],
        fin = rows[:, L], and num_tokens = #valid labels.

Device strategy (pure data parallel over batch, 8 rows per core):
  * positions laid out as pos = p*64 + f  (partition p holds 64 consecutive
    positions per row -> fully contiguous DMA of log_probs)
  * one-hot of labels (48 classes, bf16) built with DVE is_equal
  * "previous valid label" via encoded running max:
        enc = (pos*64 + label) * valid    (0 = "nothing yet")
    in-partition prefix scan with tensor_tensor_scan(max), cross-partition
    carry via PE transpose + scan + shifted transpose back;
    prev_label = running_max_exclusive mod 64
  * everything accumulates into ONE [48, 2*48] PSUM through 512 matmuls:
        psum[c1, 0, c2] += sum_pos onehot[pos,c1] * lp_bf16[pos,c2]
        psum[c1, 1, c3] += sum_pos onehot[pos,c1] * onehot_prev[pos,c3]
    trace of block 0 = emission sum;  block 1 = transition pair counts
  * A_scores log-softmax computed on device ([43, 43+pad] table);
    final dot products + first/last-label terms assembled into a [128, 4]
    column tile, reduced with a single ones-matmul -> out[4] per core:
        out = [main_score, start_score, fin_score, num_tokens]
  * host: loss = sum_cores(out0+out1+out2) / sum_cores(out3)
"""

import os
import sys

if "/opt/trn_rl_repo" not in sys.path:
    sys.path.insert(0, "/opt/trn_rl_repo")

# debug bisection knob: 1=prep+scans only, 2=+main loop, 3=full (default)
KSTAGE = int(os.environ.get("KSTAGE", "3"))

import numpy as np
import ml_dtypes

import concourse.bass as bass
import concourse.tile as tile
from concourse import bacc, mybir
from concourse.bass_utils import run_bass_kernel_spmd

# Problem dims (hardcoded per contract)
B, S, C = 64, 8192, 48
L = 42
IGNORE = -100
N_CORES = 8
B_LOC = B // N_CORES  # 8 rows per core
P = 128               # partitions
F = S // P            # 64 positions per partition per row
BIG = float(1 << 23)  # sentinel for min-scan; exact in fp32, BIG % 64 == 0

f32 = mybir.dt.float32
bf16 = mybir.dt.bfloat16
i32 = mybir.dt.int32
Alu = mybir.AluOpType
Act = mybir.ActivationFunctionType
Axis = mybir.AxisListType

_PROGRAM_CACHE = {}


def _host_constants():
    """Data-independent constant tables shipped to each core."""
    # one-hot comparisons are against label values 1..48, so onehot index
    # c corresponds to STATE c = label-1
    iota48 = np.broadcast_to(
        np.arange(1, 49, dtype=np.float32), (P, 48)
    ).astype(ml_dtypes.bfloat16)
    # class-major expanded iota for 2x-mode one-hot builds:
    # iota_exp[p, c, f] = c+1
    iota_exp = np.broadcast_to(
        np.arange(1, 49, dtype=np.float32)[None, :, None], (P, 48, F)
    ).astype(ml_dtypes.bfloat16)
    # value (p*64 + f) * 64  at [p, f]
    pos = (np.arange(P)[:, None] * F + np.arange(F)[None, :]) * 64
    iota_pos = pos.astype(np.float32)
    id128 = np.eye(P, dtype=np.float32)
    ones128 = np.ones((P, 1), dtype=np.float32)
    # emission diag selector: psum[c1, 0, c2] pairs state c1 with class c2;
    # the gold class for state c1 is c1+1
    wem = np.zeros((48, 48), np.float32)
    for c1 in range(47):
        wem[c1, c1 + 1] = 1.0
    wbase = np.concatenate([wem, np.zeros((48, 48), np.float32)], axis=1)
    return {
        "iota48": np.ascontiguousarray(iota48),
        "iota_exp": np.ascontiguousarray(iota_exp),
        "iota_pos": np.ascontiguousarray(iota_pos),
        "id128": id128,
        "ones128": ones128,
        "wbase": wbase,
    }


def build_program():
    """Build the per-core Bass/Tile program (SPMD; every core runs this)."""
    nc = bacc.Bacc("TRN2")

    lp_d = nc.declare_dram_parameter("lp", [B_LOC, S, C], f32, isOutput=False)
    lab_d = nc.declare_dram_parameter("labels", [P, B_LOC, F], i32, isOutput=False)
    a_d = nc.declare_dram_parameter("a_scores", [L + L * (L + 1)], f32, isOutput=False)
    iota48_d = nc.declare_dram_parameter("iota48", [P, 48], bf16, isOutput=False)
    iotax_d = nc.declare_dram_parameter("iota_exp", [P, 48, F], bf16, isOutput=False)
    iotap_d = nc.declare_dram_parameter("iota_pos", [P, F], f32, isOutput=False)
    id128_d = nc.declare_dram_parameter("id128", [P, P], f32, isOutput=False)
    ones_d = nc.declare_dram_parameter("ones128", [P, 1], f32, isOutput=False)
    wbase_d = nc.declare_dram_parameter("wbase", [48, 96], f32, isOutput=False)
    out_d = nc.declare_dram_parameter("out", [4], f32, isOutput=True)

    with tile.TileContext(nc) as tc:
        with (
            tc.tile_pool(name="const", bufs=1) as cpool,
            tc.tile_pool(name="lab", bufs=1) as lpool,
            tc.tile_pool(name="lp", bufs=3) as lppool,
            tc.tile_pool(name="rhs", bufs=3) as rhspool,
            tc.tile_pool(name="ohn", bufs=3) as ohnpool,
            tc.tile_pool(name="prev", bufs=3) as prevpool,
            tc.tile_pool(name="psum", bufs=1, space=bass.MemorySpace.PSUM) as ppool,
        ):
            # ---------------- constants in ----------------
            # labels first: everything else queues behind it on this ring
            lab = lpool.tile([P, B_LOC, F], i32, tag="lab")
            nc.sync.dma_start(lab[:], lab_d[:])
            iota48 = cpool.tile([P, 48], bf16, tag="iota48")
            nc.scalar.dma_start(iota48[:], iota48_d[:])
            iotax = cpool.tile([P, 48, F], bf16, tag="iotax")
            nc.scalar.dma_start(iotax[:], iotax_d[:])
            iotap = cpool.tile([P, F], f32, tag="iotap")
            nc.scalar.dma_start(iotap[:], iotap_d[:])
            id128 = cpool.tile([P, P], f32, tag="id128")
            nc.scalar.dma_start(id128[:], id128_d[:])
            ones = cpool.tile([P, 1], f32, tag="ones")
            nc.scalar.dma_start(ones[:], ones_d[:])
            W = cpool.tile([48, 96], f32, tag="W")
            nc.scalar.dma_start(W[:], wbase_d[:])

            # A-scores table: [43 states, 48] (padded with -1e30)
            table = cpool.tile([43, 48], f32, tag="table")
            nc.vector.memset(table[:], -1.0e30)
            nc.scalar.dma_start(table[0:1, 0:L], a_d[0:L].unsqueeze(0))
            nc.scalar.dma_start(
                table[1:43, 0 : L + 1],
                a_d[L:].rearrange("(r c) -> r c", r=L),
            )


            # ---------------- label prep (DVE) ----------------
            labbf = lpool.tile([P, B_LOC, F], bf16, tag="labbf")
            nc.vector.tensor_copy(labbf[:], lab[:])
            validf = lpool.tile([P, B_LOC, F], f32, tag="validf")
            nc.vector.tensor_scalar(validf[:], lab[:], 0.0, None, op0=Alu.is_gt)
            encb = lpool.tile([P, B_LOC, F], f32, tag="encb")
            iotap_b = iotap[:].unsqueeze(1).broadcast_to([P, B_LOC, F])
            nc.vector.tensor_tensor(encb[:], lab[:], iotap_b, op=Alu.add)
            enc = lpool.tile([P, B_LOC, F], f32, tag="enc")
            nc.vector.tensor_tensor(enc[:], encb[:], validf[:], op=Alu.mult)
            # label-free encoding enc0 = pos*64*valid: same running argmax as
            # enc (position-monotone), so label = enc - enc0 after any scan.
            # (HW has no mod ALU op; this replaces "enc mod 64" decodes.)
            enc0 = lpool.tile([P, B_LOC, F], f32, tag="enc0")
            nc.vector.tensor_tensor(enc0[:], iotap_b, validf[:], op=Alu.mult)

            # ---------------- scans ----------------
            # scano[:, r, 0] = 0; scano[:, r, 1+k] = max(enc[:, r, 0..k])
            scano = lpool.tile([P, B_LOC, F + 1], f32, tag="scano")
            nc.vector.memset(scano[:, :, 0:1], 0.0)
            scano0 = lpool.tile([P, B_LOC, F + 1], f32, tag="scano0")
            nc.vector.memset(scano0[:, :, 0:1], 0.0)
            for r in range(B_LOC):
                nc.vector.tensor_tensor_scan(
                    scano[:, r, 1 : F + 1],
                    enc[:, r, :],
                    enc[:, r, :],
                    0.0,
                    op0=Alu.max,
                    op1=Alu.max,
                )
                nc.vector.tensor_tensor_scan(
                    scano0[:, r, 1 : F + 1],
                    enc0[:, r, :],
                    enc0[:, r, :],
                    0.0,
                    op0=Alu.max,
                    op1=Alu.max,
                )
            # col groups at 0 / 32 / 64 so the transposed rows are 32-aligned
            # (DVE ops only accept 32-aligned start partitions)
            stats = lpool.tile([P, 96], f32, tag="stats")
            nc.vector.tensor_copy(stats[:, 0:B_LOC], scano[:, :, F])
            nc.vector.tensor_copy(stats[:, 8:16], scano0[:, :, F])
            # critical-path transpose: per-partition running maxima only
            pstatsA = ppool.tile([16, P], f32, tag="pstatsA")
            nc.tensor.transpose(pstatsA[:], stats[:, 0:16], id128[:])
            # EXCLUSIVE running max of per-partition maxima, per row
            # (rows 0..7: enc; rows 8..15: enc0): scanT[r, p] = max part < p
            # (data0 reads PSUM directly; op1=bypass ignores data1)
            scanT = lpool.tile([16, P], f32, tag="scanT")
            nc.vector.memset(scanT[:, 0:1], 0.0)
            nc.vector.tensor_tensor_scan(
                scanT[:, 1:P],
                pstatsA[0:16, 0 : P - 1],
                id128[0:16, 0 : P - 1],
                0.0,
                op0=Alu.max,
                op1=Alu.bypass,
            )
            # back into [128, 16] per-partition carry
            pP = ppool.tile([P, 16], f32, tag="pP")
            nc.tensor.transpose(pP[:], scanT[:], id128[0:16, 0:16])

            if KSTAGE >= 2:
                # ---------------- main streaming loop ----------------
                pacc = ppool.tile([48, 2, 48], f32, tag="pacc")
                for r in range(B_LOC):
                    lp_t = lppool.tile([P, F, C], f32, tag="lp_t")
                    nc.sync.dma_start(
                        lp_t[:], lp_d[r].rearrange("(p f) c -> p f c", p=P)
                    )
                    rhs_t = rhspool.tile([P, 2, F, C], bf16, tag="rhs_t")
                    nc.scalar.copy(rhs_t[:, 0], lp_t[:])
                    # class-major one-hot vs expanded iota const (2x mode)
                    ohn = ohnpool.tile([P, C, F], bf16, tag="ohn")
                    nc.vector.tensor_tensor(
                        ohn[:],
                        labbf[:, r, :].unsqueeze(1).broadcast_to([P, 48, F]),
                        iotax[:],
                        op=Alu.is_equal,
                    )
                    # prev_enc = max(in-partition exclusive scan, cross-part carry)
                    prevb = prevpool.tile([P, F], f32, tag="prevb")
                    nc.vector.scalar_tensor_tensor(
                        prevb[:],
                        scano[:, r, 0:F],
                        pP[:, r : r + 1],
                        scano[:, r, 0:F],
                        op0=Alu.max,
                        op1=Alu.max,
                    )
                    prevb0 = prevpool.tile([P, F], f32, tag="prevb0")
                    nc.vector.scalar_tensor_tensor(
                        prevb0[:],
                        scano0[:, r, 0:F],
                        pP[:, 8 + r : 9 + r],
                        scano0[:, r, 0:F],
                        op0=Alu.max,
                        op1=Alu.max,
                    )
                    prevl = prevpool.tile([P, F], f32, tag="prevl")
                    nc.vector.tensor_tensor(
                        prevl[:], prevb[:], prevb0[:], op=Alu.subtract
                    )
                    # ACT expands prev labels so the is_equal runs 2x packed
                    pexp = prevpool.tile([P, F, C], bf16, tag="pexp")
                    nc.scalar.copy(
                        pexp[:], prevl[:].unsqueeze(2).broadcast_to([P, F, 48])
                    )
                    nc.vector.tensor_tensor(
                        rhs_t[:, 1],
                        pexp[:],
                        iota48[:].unsqueeze(1).broadcast_to([P, F, 48]),
                        op=Alu.is_equal,
                    )
                    for j in range(F):
                        nc.tensor.matmul(
                            pacc[:],
                            ohn[:, :, j],
                            rhs_t[:, :, j, :],
                            start=(r == 0 and j == 0),
                            stop=(r == B_LOC - 1 and j == F - 1),
                        )

            # ---------------- deferred stats (tail-only) ----------------
            encpb = lpool.tile([P, B_LOC, F], f32, tag="encpb")
            nc.vector.tensor_scalar(encpb[:], enc[:], BIG, None, op0=Alu.add)
            encmin = lpool.tile([P, B_LOC, F], f32, tag="encmin")
            nc.vector.scalar_tensor_tensor(
                encmin[:], validf[:], -BIG, encpb[:], op0=Alu.mult, op1=Alu.add
            )
            enc0pb = lpool.tile([P, B_LOC, F], f32, tag="enc0pb")
            nc.vector.tensor_scalar(enc0pb[:], enc0[:], BIG, None, op0=Alu.add)
            encmin0 = lpool.tile([P, B_LOC, F], f32, tag="encmin0")
            nc.vector.scalar_tensor_tensor(
                encmin0[:], validf[:], -BIG, enc0pb[:], op0=Alu.mult, op1=Alu.add
            )
            nc.vector.tensor_reduce(
                stats[:, 32:40], encmin[:], axis=Axis.X, op=Alu.min
            )
            nc.vector.tensor_reduce(
                stats[:, 40:48], encmin0[:], axis=Axis.X, op=Alu.min
            )
            nc.vector.tensor_reduce(
                stats[:, 64:72], validf[:], axis=Axis.X, op=Alu.add
            )
            pstats = ppool.tile([96, P], f32, tag="pstats")
            nc.tensor.transpose(pstats[:], stats[:, 0:96], id128[:])

            # ---------------- A-scores log-softmax ----------------
            tmax = lpool.tile([43, 1], f32, tag="tmax")
            nc.vector.tensor_reduce(tmax[:], table[:], axis=Axis.X, op=Alu.max)
            x1 = lpool.tile([43, 48], f32, tag="x1")
            nc.vector.tensor_scalar(x1[:], table[:], tmax[:], None, op0=Alu.subtract)
            ex = lpool.tile([43, 48], f32, tag="ex")
            nc.scalar.activation(ex[:], x1[:], Act.Exp)
            ssum = lpool.tile([43, 1], f32, tag="ssum")
            nc.vector.tensor_reduce(ssum[:], ex[:], axis=Axis.X, op=Alu.add)
            lsum = lpool.tile([43, 1], f32, tag="lsum")
            nc.scalar.activation(lsum[:], ssum[:], Act.Ln)
            lse = lpool.tile([43, 1], f32, tag="lse")
            nc.vector.tensor_tensor(lse[:], tmax[:], lsum[:], op=Alu.add)
            tls = lpool.tile([43, 48], f32, tag="tls")
            nc.vector.tensor_scalar(tls[:], table[:], lse[:], None, op0=Alu.subtract)
            # ptT[j, i] = tls[i, j]
            ptT = ppool.tile([43, 43], f32, tag="ptT")
            nc.tensor.transpose(ptT[:], tls[0:43, 0:43], id128[0:43, 0:43])
            # W[c1, 48+c3] = trans[state c3 -> state c1] = tls[c3+1, c1]
            nc.vector.tensor_copy(W[0:42, 48:90], ptT[0:42, 1:43])
            # finrow[0, i] = tls[i, 42]; fin[state c] = finrow[0, c+1]
            finrow = ppool.tile([1, 43], f32, tag="finrow")
            nc.tensor.transpose(finrow[:], tls[0:43, 42:43], id128[0:43, 0:43])
            finrow_sb = lpool.tile([1, 43], f32, tag="finrow_sb")
            nc.vector.tensor_copy(finrow_sb[:], finrow[:])

            if KSTAGE >= 3:
                # ---------------- tail ----------------
                psb = lpool.tile([48, 96], f32, tag="psb")
                nc.vector.tensor_copy(psb[:], pacc[:].rearrange("a b c -> a (b c)"))
                Z = lpool.tile([P, 4], f32, tag="Z")
                nc.vector.memset(Z[:], 0.0)
                scratch = lpool.tile([48, 96], f32, tag="scratch")
                nc.vector.tensor_tensor(scratch[:], psb[:], W[:], op=Alu.mult)
                nc.vector.tensor_reduce(
                    Z[0:48, 0:1], scratch[:], axis=Axis.X, op=Alu.add
                )
                # first/last valid labels: pack enc/enc0 pairs into one column,
                # transpose to the free dim, subtract -> labels, one-hot, dot.
                colv = lpool.tile([P, 1], f32, tag="colv")
                nc.vector.memset(colv[:], 0.0)
                # inclusive full-row max = max(exclusive scan end, last partition)
                nc.vector.tensor_tensor(
                    colv[0:16, 0:1],
                    scanT[:, P - 1 : P],
                    pstatsA[0:16, P - 1 : P],
                    op=Alu.max,
                )
                nc.vector.tensor_reduce(
                    colv[32:48, 0:1], pstats[32:48, :], axis=Axis.X, op=Alu.min
                )
                pcv = ppool.tile([1, P], f32, tag="pcv")
                nc.tensor.transpose(pcv[:], colv[:], id128[:])
                rowT = lpool.tile([1, P], f32, tag="rowT")
                nc.vector.tensor_copy(rowT[:], pcv[:])
                ldF = lpool.tile([1, 8], f32, tag="ldF")
                nc.vector.tensor_tensor(
                    ldF[:], rowT[0:1, 0:8], rowT[0:1, 8:16], op=Alu.subtract
                )
                fdF = lpool.tile([1, 8], f32, tag="fdF")
                nc.vector.tensor_tensor(
                    fdF[:], rowT[0:1, 32:40], rowT[0:1, 40:48], op=Alu.subtract
                )
                iota42r = iota48[0:1, 0:42].unsqueeze(1).broadcast_to([1, 8, 42])
                ohf = lpool.tile([1, 8, 42], f32, tag="ohf")
                nc.vector.tensor_tensor(
                    ohf[:],
                    fdF[:].unsqueeze(2).broadcast_to([1, 8, 42]),
                    iota42r,
                    op=Alu.is_equal,
                )
                ohl = lpool.tile([1, 8, 42], f32, tag="ohl")
                nc.vector.tensor_tensor(
                    ohl[:],
                    ldF[:].unsqueeze(2).broadcast_to([1, 8, 42]),
                    iota42r,
                    op=Alu.is_equal,
                )
                sd = lpool.tile([1, 8, 42], f32, tag="sd")
                nc.vector.tensor_tensor(
                    sd[:],
                    ohf[:],
                    tls[0:1, 0:42].unsqueeze(1).broadcast_to([1, 8, 42]),
                    op=Alu.mult,
                )
                nc.vector.tensor_reduce(
                    Z[0:1, 1:2], sd[:], axis=Axis.XY, op=Alu.add
                )
                fd = lpool.tile([1, 8, 42], f32, tag="fd")
                nc.vector.tensor_tensor(
                    fd[:],
                    ohl[:],
                    finrow_sb[0:1, 1:43].unsqueeze(1).broadcast_to([1, 8, 42]),
                    op=Alu.mult,
                )
                nc.vector.tensor_reduce(
                    Z[0:1, 2:3], fd[:], axis=Axis.XY, op=Alu.add
                )
                nc.vector.tensor_reduce(
                    Z[64:72, 3:4], pstats[64:72, :], axis=Axis.X, op=Alu.add
                )
                pout = ppool.tile([4, 1], f32, tag="pout")
                nc.tensor.matmul(pout[:], Z[:], ones[:], start=True, stop=True)
                outsb = lpool.tile([4, 1], f32, tag="outsb")
                nc.vector.tensor_copy(outsb[:], pout[:])
                nc.sync.dma_start(out_d[:], outsb[:])
            else:
                outsb = lpool.tile([4, 1], f32, tag="outsb")
                if KSTAGE >= 2:
                    psb = lpool.tile([48, 96], f32, tag="psb")
                    nc.vector.tensor_copy(psb[:], pacc[:].rearrange("a b c -> a (b c)"))
                    nc.vector.tensor_copy(outsb[:], psb[0:4, 0:1])
                else:
                    nc.vector.tensor_copy(outsb[:], statsT[0:4, 0:1])
                nc.sync.dma_start(out_d[:], outsb[:])

    nc.finalize()
    return nc


def _get_program():
    if "nc" not in _PROGRAM_CACHE:
        _PROGRAM_CACHE["nc"] = build_program()
    return _PROGRAM_CACHE["nc"]


def make_in_maps(log_probs, A_scores, labels, input_lens):
    consts = _host_constants()
    in_maps = []
    for c in range(N_CORES):
        sl = slice(c * B_LOC, (c + 1) * B_LOC)
        # pre-permute labels to the on-chip layout [p, r, f], pos = p*64+f,
        # so the device DMA is one contiguous chunk per partition
        lab = np.ascontiguousarray(
            np.asarray(labels[sl], dtype=np.int32)
            .reshape(B_LOC, P, F)
            .transpose(1, 0, 2)
        )
        in_maps.append(
            {
                "lp": np.ascontiguousarray(log_probs[sl], dtype=np.float32),
                "labels": lab,
                "a_scores": np.ascontiguousarray(A_scores, dtype=np.float32),
                **consts,
            }
        )
    return in_maps


def combine_outputs(outs):
    num = 0.0
    tok = 0.0
    for o in outs:
        o = np.asarray(o, dtype=np.float64)
        num += o[0] + o[1] + o[2]
        tok += o[3]
    return np.float32(num / tok)


def kernel(log_probs, A_scores, labels, input_lens):
    nc = _get_program()
    in_maps = make_in_maps(log_probs, A_scores, labels, input_lens)
    res = run_bass_kernel_spmd(nc, in_maps, list(range(N_CORES)))
    return combine_outputs([res.results[c]["out"] for c in range(N_CORES)])



# revision 4
# speedup vs baseline: 1.2804x; 1.2804x over previous
"""Trainium2 Bass kernel for nn_CRFLoss (single-path CRF numerator loss).

Math (matches the reference):
  loss = ( sum_b [ emis_b + lm_b ] ) / num_tokens
  emis_b = sum over valid positions p of log_probs[b, p, labels[b,p]]
  lm_b   = start_lp[s0] + sum_t trans[s_{t-1}, s_t] + fin[s_{T-1}]
           over the sequence of valid labels (s = label - 1)

Device strategy (pure data parallel over batch, 8 rows per core):
  * positions laid out as pos = p*64 + f  (partition p holds 64 consecutive
    positions per row -> fully contiguous DMA of log_probs)
  * log_probs DMA'd with SWDGE f32->bf16 cast directly into the matmul rhs
    tile (no on-chip convert pass)
  * one-hot of labels (42 states, bf16) built with DVE is_equal (2x mode)
  * "previous valid label" via encoded running max:
        enc = (pos*64 + label) * valid    (0 = "nothing yet")
    in-partition prefix scan with tensor_tensor_scan(max), cross-partition
    carry via PE transpose + scan + shifted transpose back;
    prev_label = enc_run_max - enc0_run_max (enc0 tracks pos*64*valid)
  * everything accumulates into one PSUM tile through 512 matmuls,
    col-tiled 2x across the PE array (even j -> partitions 0..41,
    odd j -> partitions 64..105):
        psum[c1, 0, c2] += sum_pos onehot[pos,c1] * lp_bf16[pos,c2]
        psum[c1, 1, c3] += sum_pos onehot[pos,c1] * onehot_prev[pos,c3]
    trace of block 0 = emission sum;  block 1 = transition pair counts
  * A_scores log-softmax computed on device (host only re-packs A into a
    padded [43, 48] table = pure layout); interleaved with rows 0-1 so it
    is off the critical path
  * final dot products + first/last-label terms assembled into a [128, 4]
    column tile, reduced with a single ones-matmul -> out[4] per core:
        out = [main_score, start_score, fin_score, num_tokens]
  * host: loss = sum_cores(out0+out1+out2) / sum_cores(out3)
"""

import os
import sys

if "/opt/trn_rl_repo" not in sys.path:
    sys.path.insert(0, "/opt/trn_rl_repo")

# debug bisection knob: 1=prep+scans only, 2=+main loop, 3=full (default)
KSTAGE = int(os.environ.get("KSTAGE", "3"))
COLTILE = int(os.environ.get("COLTILE", "1"))
DMACAST = int(os.environ.get("DMACAST", "1"))

import numpy as np
import ml_dtypes

import concourse.bass as bass
import concourse.tile as tile
from concourse import bacc, mybir
from concourse.bass_utils import run_bass_kernel_spmd

# Problem dims (hardcoded per contract)
B, S, C = 64, 8192, 48
L = 42
IGNORE = -100
N_CORES = 8
B_LOC = B // N_CORES  # 8 rows per core
P = 128               # partitions
F = S // P            # 64 positions per partition per row
BIG = float(1 << 23)  # sentinel for min-scan; exact in fp32, BIG % 64 == 0

f32 = mybir.dt.float32
bf16 = mybir.dt.bfloat16
i32 = mybir.dt.int32
Alu = mybir.AluOpType
Act = mybir.ActivationFunctionType
Axis = mybir.AxisListType

# const blob layouts
BF_IOTA48 = 0          # [P, 48]   values 1..48
BF_IOTAX = 48          # [P, 42*64] class-major expanded iota (c+1)
BF_TOT = 48 + L * F    # 2736

F32_IOTAP = 0          # [P, 64]   (p*64 + f) * 64
F32_ID128 = 64         # [P, 128]  identity
F32_ONES = 192         # [P, 1]    ones
F32_W = 193            # [P, 96]   emission selector (rows 0:42 and 64:106)
F32_TABLE = 289        # [43, 48]  padded A-scores table
F32_TOT = 337

_PROGRAM_CACHE = {}


def _host_constants():
    """Data-independent constant tables shipped to each core (2 blobs)."""
    blob16 = np.zeros((P, BF_TOT), dtype=ml_dtypes.bfloat16)
    blob16[:, BF_IOTA48:BF_IOTA48 + 48] = np.arange(1, 49, dtype=np.float32)
    # iota_exp[p, c, f] = c+1 (class-major, contiguous f)
    iotax = np.broadcast_to(
        np.arange(1, L + 1, dtype=np.float32)[None, :, None], (P, L, F)
    )
    blob16[:, BF_IOTAX:] = iotax.reshape(P, L * F).astype(ml_dtypes.bfloat16)

    blob32 = np.zeros((P, F32_TOT), dtype=np.float32)
    pos = (np.arange(P)[:, None] * F + np.arange(F)[None, :]) * 64
    blob32[:, F32_IOTAP:F32_IOTAP + F] = pos.astype(np.float32)
    blob32[:, F32_ID128:F32_ID128 + P] = np.eye(P, dtype=np.float32)
    blob32[:, F32_ONES] = 1.0
    # emission diag selector: psum[c1, 0, c2] pairs state c1 with class c2;
    # the gold class for state c1 is c1+1.  Replicated at partitions 64..105
    # for the second col-tile's accumulator.
    wem = np.zeros((P, 96), np.float32)
    for c1 in range(L):
        wem[c1, c1 + 1] = 1.0
        wem[64 + c1, c1 + 1] = 1.0
    blob32[:, F32_W:F32_W + 96] = wem
    return {"blob16": blob16, "blob32": blob32}


def _pack_a_table(A_scores):
    """Pure layout: pad A into the [43, 48] per-state table (no math)."""
    t = np.full((P, 48), -1.0e30, dtype=np.float32)
    t[0, 0:L] = A_scores[0:L]
    t[1:L + 1, 0:L + 1] = A_scores[L:].reshape(L, L + 1)
    t[L + 1:, :] = 0.0
    return t


def build_program():
    """Build the per-core Bass/Tile program (SPMD; every core runs this)."""
    nc = bacc.Bacc("TRN2")

    lp_d = nc.declare_dram_parameter("lp", [B_LOC, S, C], f32, isOutput=False)
    lab_d = nc.declare_dram_parameter("labels", [P, B_LOC, F], i32, isOutput=False)
    b16_d = nc.declare_dram_parameter("blob16", [P, BF_TOT], bf16, isOutput=False)
    b32_d = nc.declare_dram_parameter("blob32", [P, F32_TOT], f32, isOutput=False)
    out_d = nc.declare_dram_parameter("out", [4], f32, isOutput=True)

    with tile.TileContext(nc) as tc:
        with (
            tc.tile_pool(name="const", bufs=1) as cpool,
            tc.tile_pool(name="lab", bufs=1) as lpool,
            tc.tile_pool(name="rhs", bufs=3) as rhspool,
            tc.tile_pool(name="ohn", bufs=3) as ohnpool,
            tc.tile_pool(name="prev", bufs=3) as prevpool,
            tc.tile_pool(name="psum", bufs=1, space=bass.MemorySpace.PSUM) as ppool,
        ):
            # ---------------- inputs in ----------------
            lab = lpool.tile([P, B_LOC, F], i32, tag="lab")
            nc.sync.dma_start(lab[:], lab_d[:])
            blob16 = cpool.tile([P, BF_TOT], bf16, tag="blob16")
            nc.sync.dma_start(blob16[:], b16_d[:])
            blob32 = cpool.tile([P, F32_TOT], f32, tag="blob32")
            nc.sync.dma_start(blob32[:], b32_d[:])

            iota48 = blob16[:, BF_IOTA48:BF_IOTA48 + 48]
            iotax = blob16[:, BF_IOTAX:].rearrange("p (c f) -> p c f", c=L)
            iotap = blob32[:, F32_IOTAP:F32_IOTAP + F]
            id128 = blob32[:, F32_ID128:F32_ID128 + P]
            ones = blob32[:, F32_ONES:F32_ONES + 1]
            W = blob32[:, F32_W:F32_W + 96]
            table = blob32[0:43, F32_TABLE:F32_TABLE + 48]

            # ---------------- label prep (DVE) ----------------
            labbf = lpool.tile([P, B_LOC, F], bf16, tag="labbf")
            nc.vector.tensor_copy(labbf[:], lab[:])
            validf = lpool.tile([P, B_LOC, F], f32, tag="validf")
            nc.vector.tensor_scalar(validf[:], lab[:], 0.0, None, op0=Alu.is_gt)
            encb = lpool.tile([P, B_LOC, F], f32, tag="encb")
            iotap_b = iotap.unsqueeze(1).broadcast_to([P, B_LOC, F])
            nc.vector.tensor_tensor(encb[:], lab[:], iotap_b, op=Alu.add)
            enc = lpool.tile([P, B_LOC, F], f32, tag="enc")
            nc.vector.tensor_tensor(enc[:], encb[:], validf[:], op=Alu.mult)
            # label-free encoding enc0 = pos*64*valid: same running argmax as
            # enc (position-monotone), so label = enc - enc0 after any scan.
            enc0 = lpool.tile([P, B_LOC, F], f32, tag="enc0")
            nc.vector.tensor_tensor(enc0[:], iotap_b, validf[:], op=Alu.mult)

            # ---------------- scans ----------------
            # scano[:, r, 0] = 0; scano[:, r, 1+k] = max(enc[:, r, 0..k])
            scano = lpool.tile([P, B_LOC, F + 1], f32, tag="scano")
            nc.vector.memset(scano[:, :, 0:1], 0.0)
            scano0 = lpool.tile([P, B_LOC, F + 1], f32, tag="scano0")
            nc.vector.memset(scano0[:, :, 0:1], 0.0)
            for r in range(B_LOC):
                nc.vector.tensor_tensor_scan(
                    scano[:, r, 1 : F + 1],
                    enc[:, r, :],
                    enc[:, r, :],
                    0.0,
                    op0=Alu.max,
                    op1=Alu.max,
                )
                nc.vector.tensor_tensor_scan(
                    scano0[:, r, 1 : F + 1],
                    enc0[:, r, :],
                    enc0[:, r, :],
                    0.0,
                    op0=Alu.max,
                    op1=Alu.max,
                )
            # col groups at 0 / 32 / 64 so the transposed rows are 32-aligned
            # (DVE ops only accept 32-aligned start partitions)
            stats = lpool.tile([P, 96], f32, tag="stats")
            nc.vector.tensor_copy(stats[:, 0:B_LOC], scano[:, :, F])
            nc.vector.tensor_copy(stats[:, 8:16], scano0[:, :, F])
            # critical-path transpose: per-partition running maxima only
            pstatsA = ppool.tile([16, P], f32, tag="pstatsA")
            nc.tensor.transpose(pstatsA[:], stats[:, 0:16], id128)
            # EXCLUSIVE running max of per-partition maxima, per row
            # (rows 0..7: enc; rows 8..15: enc0): scanT[r, p] = max part < p
            scanT = lpool.tile([16, P], f32, tag="scanT")
            nc.vector.memset(scanT[:, 0:1], 0.0)
            nc.vector.tensor_tensor_scan(
                scanT[:, 1:P],
                pstatsA[0:16, 0 : P - 1],
                id128[0:16, 0 : P - 1],
                0.0,
                op0=Alu.max,
                op1=Alu.bypass,
            )
            # back into [128, 16] per-partition carry
            pP = ppool.tile([P, 16], f32, tag="pP")
            nc.tensor.transpose(pP[:], scanT[:], id128[0:16, 0:16])

            # A-scores log-softmax pieces (emitted interleaved with rows 0-1
            # below so neither DVE nor ACT stalls at the head of the FIFO)
            tmax = lpool.tile([43, 1], f32, tag="tmax")
            x1 = lpool.tile([43, 48], f32, tag="x1")
            ex = lpool.tile([43, 48], f32, tag="ex")
            ssum = lpool.tile([43, 1], f32, tag="ssum")
            lsum = lpool.tile([43, 1], f32, tag="lsum")
            lse = lpool.tile([43, 1], f32, tag="lse")
            tls = lpool.tile([43, 48], f32, tag="tls")
            ptT = ppool.tile([43, 43], f32, tag="ptT")
            finrow = ppool.tile([1, 43], f32, tag="finrow")
            finrow_sb = lpool.tile([1, 43], f32, tag="finrow_sb")

            def softmax_part1():
                nc.vector.tensor_reduce(tmax[:], table, axis=Axis.X, op=Alu.max)
                nc.vector.tensor_scalar(x1[:], table, tmax[:], None, op0=Alu.subtract)
                nc.scalar.activation(ex[:], x1[:], Act.Exp)

            def softmax_part2():
                nc.vector.tensor_reduce(ssum[:], ex[:], axis=Axis.X, op=Alu.add)
                nc.scalar.activation(lsum[:], ssum[:], Act.Ln)
                nc.vector.tensor_tensor(lse[:], tmax[:], lsum[:], op=Alu.add)
                nc.vector.tensor_scalar(tls[:], table, lse[:], None, op0=Alu.subtract)
                # ptT[j, i] = tls[i, j]
                nc.tensor.transpose(ptT[:], tls[0:43, 0:43], id128[0:43, 0:43])
                # W[c1, 48+c3] = trans[state c3 -> state c1] = tls[c3+1, c1]
                nc.vector.tensor_copy(W[0:L, 48:48 + L], ptT[0:L, 1:43])
                if COLTILE:
                    # second col-tile's accumulator needs W at partitions 64+
                    nc.scalar.dma_start(W[64:64 + L, 48:48 + L], W[0:L, 48:48 + L])
                # finrow[0, i] = tls[i, 42]; fin[state c] = finrow[0, c+1]
                nc.tensor.transpose(finrow[:], tls[0:43, 42:43], id128[0:43, 0:43])
                nc.vector.tensor_copy(finrow_sb[:], finrow[:])

            if KSTAGE >= 2:
                # ---------------- main streaming loop ----------------
                # accumulator: even j -> rows 0..41, odd j -> rows 64..105
                pacc = ppool.tile([106 if COLTILE else L, 2, 48], f32, tag="pacc")
                for r in range(B_LOC):
                    rhs_t = rhspool.tile([P, 2, F, C], bf16, tag="rhs_t")
                    if DMACAST:
                        # SWDGE casts f32->bf16 in the DMA datapath
                        nc.gpsimd.dma_start(
                            rhs_t[:, 0], lp_d[r].rearrange("(p f) c -> p f c", p=P)
                        )
                    else:
                        lp_t = rhspool.tile([P, F, C], f32, tag="lp_t")
                        nc.sync.dma_start(
                            lp_t[:], lp_d[r].rearrange("(p f) c -> p f c", p=P)
                        )
                        nc.scalar.copy(rhs_t[:, 0], lp_t[:])
                    # class-major one-hot vs expanded iota const (2x mode)
                    ohn = ohnpool.tile([P, L, F], bf16, tag="ohn")
                    nc.vector.tensor_tensor(
                        ohn[:],
                        labbf[:, r, :].unsqueeze(1).broadcast_to([P, L, F]),
                        iotax,
                        op=Alu.is_equal,
                    )
                    # prev_enc = max(in-partition exclusive scan, cross-part carry)
                    prevb = prevpool.tile([P, F], f32, tag="prevb")
                    nc.vector.scalar_tensor_tensor(
                        prevb[:],
                        scano[:, r, 0:F],
                        pP[:, r : r + 1],
                        scano[:, r, 0:F],
                        op0=Alu.max,
                        op1=Alu.max,
                    )
                    prevb0 = prevpool.tile([P, F], f32, tag="prevb0")
                    nc.vector.scalar_tensor_tensor(
                        prevb0[:],
                        scano0[:, r, 0:F],
                        pP[:, 8 + r : 9 + r],
                        scano0[:, r, 0:F],
                        op0=Alu.max,
                        op1=Alu.max,
                    )
                    prevl = prevpool.tile([P, F], f32, tag="prevl")
                    nc.vector.tensor_tensor(
                        prevl[:], prevb[:], prevb0[:], op=Alu.subtract
                    )
                    # ACT expands prev labels so the is_equal runs 2x packed
                    pexp = prevpool.tile([P, F, C], bf16, tag="pexp")
                    nc.scalar.copy(
                        pexp[:], prevl[:].unsqueeze(2).broadcast_to([P, F, C])
                    )
                    nc.vector.tensor_tensor(
                        rhs_t[:, 1],
                        pexp[:],
                        iota48.unsqueeze(1).broadcast_to([P, F, C]),
                        op=Alu.is_equal,
                    )
                    for j in range(F):
                        if COLTILE:
                            tgt = pacc[0:L] if j % 2 == 0 else pacc[64:64 + L]
                            first = r == 0 and j < 2
                            last = r == B_LOC - 1 and j >= F - 2
                        else:
                            tgt = pacc[0:L]
                            first = r == 0 and j == 0
                            last = r == B_LOC - 1 and j == F - 1
                        nc.tensor.matmul(
                            tgt,
                            ohn[:, :, j],
                            rhs_t[:, :, j, :],
                            start=first,
                            stop=last,
                            skip_group_check=True,
                        )
                    if r == 0:
                        softmax_part1()
                    elif r == 1:
                        softmax_part2()
            else:
                softmax_part1()
                softmax_part2()

            # ---------------- deferred stats (tail-only) ----------------
            encpb = lpool.tile([P, B_LOC, F], f32, tag="encpb")
            nc.vector.tensor_scalar(encpb[:], enc[:], BIG, None, op0=Alu.add)
            encmin = lpool.tile([P, B_LOC, F], f32, tag="encmin")
            nc.vector.scalar_tensor_tensor(
                encmin[:], validf[:], -BIG, encpb[:], op0=Alu.mult, op1=Alu.add
            )
            enc0pb = lpool.tile([P, B_LOC, F], f32, tag="enc0pb")
            nc.vector.tensor_scalar(enc0pb[:], enc0[:], BIG, None, op0=Alu.add)
            encmin0 = lpool.tile([P, B_LOC, F], f32, tag="encmin0")
            nc.vector.scalar_tensor_tensor(
                encmin0[:], validf[:], -BIG, enc0pb[:], op0=Alu.mult, op1=Alu.add
            )
            nc.vector.tensor_reduce(
                stats[:, 32:40], encmin[:], axis=Axis.X, op=Alu.min
            )
            nc.vector.tensor_reduce(
                stats[:, 40:48], encmin0[:], axis=Axis.X, op=Alu.min
            )
            nc.vector.tensor_reduce(
                stats[:, 64:72], validf[:], axis=Axis.X, op=Alu.add
            )
            pstats = ppool.tile([96, P], f32, tag="pstats")
            nc.tensor.transpose(pstats[:], stats[:, 0:96], id128)

            if KSTAGE >= 3:
                # ---------------- tail ----------------
                psb = lpool.tile([106 if COLTILE else L, 96], f32, tag="psb")
                scratch = lpool.tile([106 if COLTILE else L, 96], f32, tag="scratch")
                Z = lpool.tile([P, 4], f32, tag="Z")
                nc.vector.memset(Z[:], 0.0)
                pacc_f = pacc[:].rearrange("a b c -> a (b c)")
                nc.vector.tensor_copy(psb[0:L], pacc_f[0:L])
                nc.vector.tensor_tensor(
                    scratch[0:L], psb[0:L], W[0:L], op=Alu.mult
                )
                nc.vector.tensor_reduce(
                    Z[0:L, 0:1], scratch[0:L], axis=Axis.X, op=Alu.add
                )
                if COLTILE:
                    nc.vector.tensor_copy(psb[64:64 + L], pacc_f[64:64 + L])
                    nc.vector.tensor_tensor(
                        scratch[64:64 + L], psb[64:64 + L], W[64:64 + L],
                        op=Alu.mult,
                    )
                    nc.vector.tensor_reduce(
                        Z[64:64 + L, 1:2], scratch[64:64 + L], axis=Axis.X,
                        op=Alu.add,
                    )
                # first/last valid labels: pack enc/enc0 pairs into one column,
                # transpose to the free dim, subtract -> labels, one-hot, dot.
                colv = lpool.tile([P, 1], f32, tag="colv")
                nc.vector.memset(colv[:], 0.0)
                # inclusive full-row max = max(exclusive scan end, last partition)
                nc.vector.tensor_tensor(
                    colv[0:16, 0:1],
                    scanT[:, P - 1 : P],
                    pstatsA[0:16, P - 1 : P],
                    op=Alu.max,
                )
                nc.vector.tensor_reduce(
                    colv[32:48, 0:1], pstats[32:48, :], axis=Axis.X, op=Alu.min
                )
                pcv = ppool.tile([1, P], f32, tag="pcv")
                nc.tensor.transpose(pcv[:], colv[:], id128)
                rowT = lpool.tile([1, P], f32, tag="rowT")
                nc.vector.tensor_copy(rowT[:], pcv[:])
                ldF = lpool.tile([1, 8], f32, tag="ldF")
                nc.vector.tensor_tensor(
                    ldF[:], rowT[0:1, 0:8], rowT[0:1, 8:16], op=Alu.subtract
                )
                fdF = lpool.tile([1, 8], f32, tag="fdF")
                nc.vector.tensor_tensor(
                    fdF[:], rowT[0:1, 32:40], rowT[0:1, 40:48], op=Alu.subtract
                )
                iota42r = iota48[0:1, 0:42].unsqueeze(1).broadcast_to([1, 8, 42])
                ohf = lpool.tile([1, 8, 42], f32, tag="ohf")
                nc.vector.tensor_tensor(
                    ohf[:],
                    fdF[:].unsqueeze(2).broadcast_to([1, 8, 42]),
                    iota42r,
                    op=Alu.is_equal,
                )
                ohl = lpool.tile([1, 8, 42], f32, tag="ohl")
                nc.vector.tensor_tensor(
                    ohl[:],
                    ldF[:].unsqueeze(2).broadcast_to([1, 8, 42]),
                    iota42r,
                    op=Alu.is_equal,
                )
                sd = lpool.tile([1, 8, 42], f32, tag="sd")
                nc.vector.tensor_tensor(
                    sd[:],
                    ohf[:],
                    tls[0:1, 0:42].unsqueeze(1).broadcast_to([1, 8, 42]),
                    op=Alu.mult,
                )
                nc.vector.tensor_reduce(
                    Z[0:1, 1:2], sd[:], axis=Axis.XY, op=Alu.add
                )
                fd = lpool.tile([1, 8, 42], f32, tag="fd")
                nc.vector.tensor_tensor(
                    fd[:],
                    ohl[:],
                    finrow_sb[0:1, 1:43].unsqueeze(1).broadcast_to([1, 8, 42]),
                    op=Alu.mult,
                )
                nc.vector.tensor_reduce(
                    Z[0:1, 2:3], fd[:], axis=Axis.XY, op=Alu.add
                )
                nc.vector.tensor_reduce(
                    Z[64:72, 3:4], pstats[64:72, :], axis=Axis.X, op=Alu.add
                )
                pout = ppool.tile([4, 1], f32, tag="pout")
                nc.tensor.matmul(pout[:], Z[:], ones, start=True, stop=True)
                outsb = lpool.tile([4, 1], f32, tag="outsb")
                nc.vector.tensor_copy(outsb[:], pout[:])
                nc.sync.dma_start(out_d[:], outsb[:])
            else:
                outsb = lpool.tile([4, 1], f32, tag="outsb")
                if KSTAGE >= 2:
                    psb = lpool.tile([L, 96], f32, tag="psb")
                    nc.vector.tensor_copy(psb[:], pacc[0:L].rearrange("a b c -> a (b c)"))
                    nc.vector.tensor_copy(outsb[:], psb[0:4, 0:1])
                else:
                    nc.vector.tensor_copy(outsb[:], pstats[0:4, 0:1])
                nc.sync.dma_start(out_d[:], outsb[:])

    nc.finalize()
    return nc


def _get_program():
    if "nc" not in _PROGRAM_CACHE:
        _PROGRAM_CACHE["nc"] = build_program()
    return _PROGRAM_CACHE["nc"]


def make_in_maps(log_probs, A_scores, labels, input_lens):
    consts = _host_constants()
    atab = _pack_a_table(np.asarray(A_scores, dtype=np.float32))
    consts["blob32"] = consts["blob32"].copy()
    consts["blob32"][:, F32_TABLE:F32_TABLE + 48] = atab
    in_maps = []
    for c in range(N_CORES):
        sl = slice(c * B_LOC, (c + 1) * B_LOC)
        # pre-permute labels to the on-chip layout [p, r, f], pos = p*64+f,
        # so the device DMA is one contiguous chunk per partition
        lab = np.ascontiguousarray(
            np.asarray(labels[sl], dtype=np.int32)
            .reshape(B_LOC, P, F)
            .transpose(1, 0, 2)
        )
        in_maps.append(
            {
                "lp": np.ascontiguousarray(log_probs[sl], dtype=np.float32),
                "labels": lab,
                **consts,
            }
        )
    return in_maps


def combine_outputs(outs):
    num = 0.0
    tok = 0.0
    for o in outs:
        o = np.asarray(o, dtype=np.float64)
        num += o[0] + o[1] + o[2]
        tok += o[3]
    return np.float32(num / tok)


def kernel(log_probs, A_scores, labels, input_lens):
    nc = _get_program()
    in_maps = make_in_maps(log_probs, A_scores, labels, input_lens)
    res = run_bass_kernel_spmd(nc, in_maps, list(range(N_CORES)))
    return combine_outputs([res.results[c]["out"] for c in range(N_CORES)])


# revision 10
# speedup vs baseline: 1.4962x; 1.1685x over previous
"""Trainium2 Bass kernel for nn_CRFLoss (single-path CRF numerator loss).

Math (matches the reference):
  loss = ( sum_b [ emis_b + lm_b ] ) / num_tokens
  emis_b = sum over valid positions p of log_probs[b, p, labels[b,p]]
  lm_b   = start_lp[s0] + sum_t trans[s_{t-1}, s_t] + fin[s_{T-1}]
           over the sequence of valid labels (s = label - 1)

Device strategy (pure data parallel over batch, 8 rows per core):
  * positions laid out as pos = p*64 + f  (partition p holds 64 consecutive
    positions per row -> fully contiguous DMA of log_probs)
  * log_probs DMA'd with SWDGE f32->bf16 cast directly into the matmul rhs
    tile (no on-chip convert pass)
  * one-hot of labels (42 states, bf16) built with DVE is_equal (2x mode)
  * "previous valid label" via encoded running max:
        enc = (pos*64 + label) * valid    (0 = "nothing yet")
    in-partition prefix scan with tensor_tensor_scan(max), cross-partition
    carry via PE transpose + scan + shifted transpose back;
    prev_label = enc_run_max - enc0_run_max (enc0 tracks pos*64*valid)
  * everything accumulates into one PSUM tile through 512 matmuls,
    col-tiled 2x across the PE array (even j -> partitions 0..41,
    odd j -> partitions 64..105):
        psum[c1, 0, c2] += sum_pos onehot[pos,c1] * lp_bf16[pos,c2]
        psum[c1, 1, c3] += sum_pos onehot[pos,c1] * onehot_prev[pos,c3]
    trace of block 0 = emission sum;  block 1 = transition pair counts
  * A_scores log-softmax computed on device (host only re-packs A into a
    padded [43, 48] table = pure layout); interleaved with rows 0-1 so it
    is off the critical path
  * final dot products + first/last-label terms assembled into a [128, 4]
    column tile, reduced with a single ones-matmul -> out[4] per core:
        out = [main_score, start_score, fin_score, num_tokens]
  * host: loss = sum_cores(out0+out1+out2) / sum_cores(out3)
"""

import os
import sys

if "/opt/trn_rl_repo" not in sys.path:
    sys.path.insert(0, "/opt/trn_rl_repo")

# debug bisection knob: 1=prep+scans only, 2=+main loop, 3=full (default)
KSTAGE = int(os.environ.get("KSTAGE", "3"))
COLTILE = int(os.environ.get("COLTILE", "1"))
DMACAST = int(os.environ.get("DMACAST", "1"))
# decode prev label as (enc & 63) in int32 instead of the enc0 dual-scan
INTDECODE = int(os.environ.get("INTDECODE", "1"))
# const blobs on the gpsimd SWDGE ring ahead of lp (same-ring FIFO ordering)
CONSTGP = int(os.environ.get("CONSTGP", "1"))

import numpy as np
import ml_dtypes

import concourse.bass as bass
import concourse.tile as tile
from concourse import bacc, mybir
from concourse.bass_utils import run_bass_kernel_spmd

# Problem dims (hardcoded per contract)
B, S, C = 64, 8192, 48
L = 42
IGNORE = -100
N_CORES = 8
B_LOC = B // N_CORES  # 8 rows per core
P = 128               # partitions
F = S // P            # 64 positions per partition per row
BIG = float(1 << 23)  # sentinel for min-scan; exact in fp32, BIG % 64 == 0

f32 = mybir.dt.float32
bf16 = mybir.dt.bfloat16
i32 = mybir.dt.int32
Alu = mybir.AluOpType
Act = mybir.ActivationFunctionType
Axis = mybir.AxisListType

# const blob layouts
BF_IOTA48 = 0          # [P, 48]   values 1..48
BF_IOTAX = 48          # [P, 42*64] class-major expanded iota (c+1)
BF_TOT = 48 + L * F    # 2736

F32_IOTAP = 0          # [P, 64]   (p*64 + f) * 64
F32_ID128 = 64         # [P, 128]  identity
F32_ONES = 192         # [P, 1]    ones
F32_W = 193            # [P, 96]   emission selector (rows 0:42 and 64:106)
F32_TABLE = 289        # [43, 48]  padded A-scores table
F32_TOT = 337

_PROGRAM_CACHE = {}


def _host_constants():
    """Data-independent constant tables shipped to each core (2 blobs)."""
    blob16 = np.zeros((P, BF_TOT), dtype=ml_dtypes.bfloat16)
    blob16[:, BF_IOTA48:BF_IOTA48 + 48] = np.arange(1, 49, dtype=np.float32)
    # iota_exp[p, c, f] = c+1 (class-major, contiguous f)
    iotax = np.broadcast_to(
        np.arange(1, L + 1, dtype=np.float32)[None, :, None], (P, L, F)
    )
    blob16[:, BF_IOTAX:] = iotax.reshape(P, L * F).astype(ml_dtypes.bfloat16)

    blob32 = np.zeros((P, F32_TOT), dtype=np.float32)
    pos = (np.arange(P)[:, None] * F + np.arange(F)[None, :]) * 64
    blob32[:, F32_IOTAP:F32_IOTAP + F] = pos.astype(np.float32)
    blob32[:, F32_ID128:F32_ID128 + P] = np.eye(P, dtype=np.float32)
    blob32[:, F32_ONES] = 1.0
    # emission diag selector: psum[c1, 0, c2] pairs state c1 with class c2;
    # the gold class for state c1 is c1+1.  Replicated at partitions 64..105
    # for the second col-tile's accumulator.
    wem = np.zeros((P, 96), np.float32)
    for c1 in range(L):
        wem[c1, c1 + 1] = 1.0
        wem[64 + c1, c1 + 1] = 1.0
    blob32[:, F32_W:F32_W + 96] = wem
    return {"blob16": blob16, "blob32": blob32}


def _pack_a_table(A_scores):
    """Pure layout: pad A into the [43, 48] per-state table (no math)."""
    t = np.full((P, 48), -1.0e30, dtype=np.float32)
    t[0, 0:L] = A_scores[0:L]
    t[1:L + 1, 0:L + 1] = A_scores[L:].reshape(L, L + 1)
    t[L + 1:, :] = 0.0
    return t


def build_program():
    """Build the per-core Bass/Tile program (SPMD; every core runs this)."""
    nc = bacc.Bacc("TRN2")

    lp_d = nc.declare_dram_parameter("lp", [B_LOC, S, C], f32, isOutput=False)
    lab_d = nc.declare_dram_parameter("labels", [P, B_LOC, F], i32, isOutput=False)
    b16_d = nc.declare_dram_parameter("blob16", [P, BF_TOT], bf16, isOutput=False)
    b32_d = nc.declare_dram_parameter("blob32", [P, F32_TOT], f32, isOutput=False)
    out_d = nc.declare_dram_parameter("out", [4], f32, isOutput=True)

    with tile.TileContext(nc) as tc:
        with (
            tc.tile_pool(name="const", bufs=1) as cpool,
            tc.tile_pool(name="lab", bufs=1) as lpool,
            tc.tile_pool(name="rhs", bufs=3) as rhspool,
            tc.tile_pool(name="ohn", bufs=3) as ohnpool,
            tc.tile_pool(name="prev", bufs=3) as prevpool,
            tc.tile_pool(name="psum", bufs=1, space=bass.MemorySpace.PSUM) as ppool,
        ):
            # ---------------- inputs in ----------------
            # labels alone on the sync/HWDGE ring (fastest path, tiny);
            # const blobs lead the gpsimd/SWDGE ring so they complete before
            # the big lp streams that share it (per-ring FIFO ordering).
            lab = lpool.tile([P, B_LOC, F], i32, tag="lab")
            nc.sync.dma_start(lab[:], lab_d[:])
            cdma = nc.gpsimd if CONSTGP else nc.sync
            blob16 = cpool.tile([P, BF_TOT], bf16, tag="blob16")
            cdma.dma_start(blob16[:], b16_d[:])
            blob32 = cpool.tile([P, F32_TOT], f32, tag="blob32")
            cdma.dma_start(blob32[:], b32_d[:])

            iota48 = blob16[:, BF_IOTA48:BF_IOTA48 + 48]
            iotax = blob16[:, BF_IOTAX:].rearrange("p (c f) -> p c f", c=L)
            iotap = blob32[:, F32_IOTAP:F32_IOTAP + F]
            id128 = blob32[:, F32_ID128:F32_ID128 + P]
            ones = blob32[:, F32_ONES:F32_ONES + 1]
            W = blob32[:, F32_W:F32_W + 96]
            table = blob32[0:43, F32_TABLE:F32_TABLE + 48]

            # ---------------- label prep (DVE) ----------------
            labbf = lpool.tile([P, B_LOC, F], bf16, tag="labbf")
            nc.vector.tensor_copy(labbf[:], lab[:])
            validf = lpool.tile([P, B_LOC, F], f32, tag="validf")
            nc.vector.tensor_scalar(validf[:], lab[:], 0.0, None, op0=Alu.is_gt)
            encb = lpool.tile([P, B_LOC, F], f32, tag="encb")
            iotap_b = iotap.unsqueeze(1).broadcast_to([P, B_LOC, F])
            nc.vector.tensor_tensor(encb[:], lab[:], iotap_b, op=Alu.add)
            enc = lpool.tile([P, B_LOC, F], f32, tag="enc")
            nc.vector.tensor_tensor(enc[:], encb[:], validf[:], op=Alu.mult)
            if not INTDECODE:
                # label-free encoding enc0 = pos*64*valid: same running argmax
                # as enc (position-monotone), so label = enc - enc0 post-scan.
                enc0 = lpool.tile([P, B_LOC, F], f32, tag="enc0")
                nc.vector.tensor_tensor(enc0[:], iotap_b, validf[:], op=Alu.mult)

            # ---------------- scans ----------------
            NS = 8 if INTDECODE else 16  # carried scan rows (enc [, enc0])
            # scano[:, r, 0] = 0; scano[:, r, 1+k] = max(enc[:, r, 0..k])
            scano = lpool.tile([P, B_LOC, F + 1], f32, tag="scano")
            nc.vector.memset(scano[:, :, 0:1], 0.0)
            if not INTDECODE:
                scano0 = lpool.tile([P, B_LOC, F + 1], f32, tag="scano0")
                nc.vector.memset(scano0[:, :, 0:1], 0.0)
            for r in range(B_LOC):
                nc.vector.tensor_tensor_scan(
                    scano[:, r, 1 : F + 1],
                    enc[:, r, :],
                    enc[:, r, :],
                    0.0,
                    op0=Alu.max,
                    op1=Alu.max,
                )
                if not INTDECODE:
                    nc.vector.tensor_tensor_scan(
                        scano0[:, r, 1 : F + 1],
                        enc0[:, r, :],
                        enc0[:, r, :],
                        0.0,
                        op0=Alu.max,
                        op1=Alu.max,
                    )
            # col groups at 0 / 32 / 64 so the transposed rows are 32-aligned
            # (DVE ops only accept 32-aligned start partitions)
            stats = lpool.tile([P, 96], f32, tag="stats")
            nc.vector.tensor_copy(stats[:, 0:B_LOC], scano[:, :, F])
            if not INTDECODE:
                nc.vector.tensor_copy(stats[:, 8:16], scano0[:, :, F])
            # critical-path transpose: per-partition running maxima only
            pstatsA = ppool.tile([NS, P], f32, tag="pstatsA")
            nc.tensor.transpose(pstatsA[:], stats[:, 0:NS], id128)
            # EXCLUSIVE running max of per-partition maxima, per row
            # (rows 0..7: enc; rows 8..15: enc0): scanT[r, p] = max part < p
            scanT = lpool.tile([NS, P], f32, tag="scanT")
            nc.vector.memset(scanT[:, 0:1], 0.0)
            nc.vector.tensor_tensor_scan(
                scanT[:, 1:P],
                pstatsA[0:NS, 0 : P - 1],
                id128[0:NS, 0 : P - 1],
                0.0,
                op0=Alu.max,
                op1=Alu.bypass,
            )
            # back into [128, NS] per-partition carry
            pP = ppool.tile([P, NS], f32, tag="pP")
            nc.tensor.transpose(pP[:], scanT[:], id128[0:NS, 0:NS])

            # A-scores log-softmax pieces (emitted interleaved with rows 0-1
            # below so neither DVE nor ACT stalls at the head of the FIFO)
            tmax = lpool.tile([43, 1], f32, tag="tmax")
            x1 = lpool.tile([43, 48], f32, tag="x1")
            ex = lpool.tile([43, 48], f32, tag="ex")
            ssum = lpool.tile([43, 1], f32, tag="ssum")
            lsum = lpool.tile([43, 1], f32, tag="lsum")
            lse = lpool.tile([43, 1], f32, tag="lse")
            tls = lpool.tile([43, 48], f32, tag="tls")
            ptT = ppool.tile([43, 43], f32, tag="ptT")
            finrow = ppool.tile([1, 43], f32, tag="finrow")
            finrow_sb = lpool.tile([1, 43], f32, tag="finrow_sb")

            def softmax_part1():
                nc.vector.tensor_reduce(tmax[:], table, axis=Axis.X, op=Alu.max)
                nc.vector.tensor_scalar(x1[:], table, tmax[:], None, op0=Alu.subtract)
                nc.scalar.activation(ex[:], x1[:], Act.Exp)

            def softmax_part2():
                nc.vector.tensor_reduce(ssum[:], ex[:], axis=Axis.X, op=Alu.add)
                nc.scalar.activation(lsum[:], ssum[:], Act.Ln)
                nc.vector.tensor_tensor(lse[:], tmax[:], lsum[:], op=Alu.add)
                nc.vector.tensor_scalar(tls[:], table, lse[:], None, op0=Alu.subtract)
                # ptT[j, i] = tls[i, j]
                nc.tensor.transpose(ptT[:], tls[0:43, 0:43], id128[0:43, 0:43])
                # W[c1, 48+c3] = trans[state c3 -> state c1] = tls[c3+1, c1]
                nc.vector.tensor_copy(W[0:L, 48:48 + L], ptT[0:L, 1:43])
                if COLTILE:
                    # second col-tile's accumulator needs W at partitions 64+
                    nc.scalar.dma_start(W[64:64 + L, 48:48 + L], W[0:L, 48:48 + L])
                # finrow[0, i] = tls[i, 42]; fin[state c] = finrow[0, c+1]
                nc.tensor.transpose(finrow[:], tls[0:43, 42:43], id128[0:43, 0:43])
                nc.vector.tensor_copy(finrow_sb[:], finrow[:])

            if KSTAGE >= 2:
                # ---------------- main streaming loop ----------------
                # accumulator: even j -> rows 0..41, odd j -> rows 64..105
                pacc = ppool.tile([106 if COLTILE else L, 2, 48], f32, tag="pacc")
                for r in range(B_LOC):
                    rhs_t = rhspool.tile([P, 2, F, C], bf16, tag="rhs_t")
                    if DMACAST:
                        # SWDGE casts f32->bf16 in the DMA datapath
                        nc.gpsimd.dma_start(
                            rhs_t[:, 0], lp_d[r].rearrange("(p f) c -> p f c", p=P)
                        )
                    else:
                        lp_t = rhspool.tile([P, F, C], f32, tag="lp_t")
                        nc.sync.dma_start(
                            lp_t[:], lp_d[r].rearrange("(p f) c -> p f c", p=P)
                        )
                        nc.scalar.copy(rhs_t[:, 0], lp_t[:])
                    # class-major one-hot vs expanded iota const (2x mode)
                    ohn = ohnpool.tile([P, L, F], bf16, tag="ohn")
                    nc.vector.tensor_tensor(
                        ohn[:],
                        labbf[:, r, :].unsqueeze(1).broadcast_to([P, L, F]),
                        iotax,
                        op=Alu.is_equal,
                    )
                    # prev_enc = max(in-partition exclusive scan, cross-part carry)
                    prevb = prevpool.tile([P, F], f32, tag="prevb")
                    nc.vector.scalar_tensor_tensor(
                        prevb[:],
                        scano[:, r, 0:F],
                        pP[:, r : r + 1],
                        scano[:, r, 0:F],
                        op0=Alu.max,
                        op1=Alu.max,
                    )
                    if INTDECODE:
                        # prev label = enc & 63 (positions encoded as pos*64)
                        previ = prevpool.tile([P, F], i32, tag="previ")
                        nc.vector.tensor_copy(previ[:], prevb[:])
                        prevm = prevpool.tile([P, F], i32, tag="prevm")
                        nc.vector.tensor_scalar(
                            prevm[:], previ[:], 63, None, op0=Alu.bitwise_and
                        )
                        prevl = prevpool.tile([P, F], f32, tag="prevl")
                        nc.vector.tensor_copy(prevl[:], prevm[:])
                    else:
                        prevb0 = prevpool.tile([P, F], f32, tag="prevb0")
                        nc.vector.scalar_tensor_tensor(
                            prevb0[:],
                            scano0[:, r, 0:F],
                            pP[:, 8 + r : 9 + r],
                            scano0[:, r, 0:F],
                            op0=Alu.max,
                            op1=Alu.max,
                        )
                        prevl = prevpool.tile([P, F], f32, tag="prevl")
                        nc.vector.tensor_tensor(
                            prevl[:], prevb[:], prevb0[:], op=Alu.subtract
                        )
                    # ACT expands prev labels so the is_equal runs 2x packed
                    pexp = prevpool.tile([P, F, C], bf16, tag="pexp")
                    nc.scalar.copy(
                        pexp[:], prevl[:].unsqueeze(2).broadcast_to([P, F, C])
                    )
                    nc.vector.tensor_tensor(
                        rhs_t[:, 1],
                        pexp[:],
                        iota48.unsqueeze(1).broadcast_to([P, F, C]),
                        op=Alu.is_equal,
                    )
                    for j in range(F):
                        if COLTILE:
                            tgt = pacc[0:L] if j % 2 == 0 else pacc[64:64 + L]
                            first = r == 0 and j < 2
                            last = r == B_LOC - 1 and j >= F - 2
                        else:
                            tgt = pacc[0:L]
                            first = r == 0 and j == 0
                            last = r == B_LOC - 1 and j == F - 1
                        nc.tensor.matmul(
                            tgt,
                            ohn[:, :, j],
                            rhs_t[:, :, j, :],
                            start=first,
                            stop=last,
                            skip_group_check=True,
                        )
                    if r == 0:
                        softmax_part1()
                    elif r == 1:
                        softmax_part2()
            else:
                softmax_part1()
                softmax_part2()

            # ---------------- deferred stats (tail-only) ----------------
            encpb = lpool.tile([P, B_LOC, F], f32, tag="encpb")
            nc.vector.tensor_scalar(encpb[:], enc[:], BIG, None, op0=Alu.add)
            encmin = lpool.tile([P, B_LOC, F], f32, tag="encmin")
            nc.vector.scalar_tensor_tensor(
                encmin[:], validf[:], -BIG, encpb[:], op0=Alu.mult, op1=Alu.add
            )
            nc.vector.tensor_reduce(
                stats[:, 32:40], encmin[:], axis=Axis.X, op=Alu.min
            )
            if not INTDECODE:
                enc0pb = lpool.tile([P, B_LOC, F], f32, tag="enc0pb")
                nc.vector.tensor_scalar(enc0pb[:], enc0[:], BIG, None, op0=Alu.add)
                encmin0 = lpool.tile([P, B_LOC, F], f32, tag="encmin0")
                nc.vector.scalar_tensor_tensor(
                    encmin0[:], validf[:], -BIG, enc0pb[:], op0=Alu.mult, op1=Alu.add
                )
                nc.vector.tensor_reduce(
                    stats[:, 40:48], encmin0[:], axis=Axis.X, op=Alu.min
                )
            nc.vector.tensor_reduce(
                stats[:, 64:72], validf[:], axis=Axis.X, op=Alu.add
            )
            pstats = ppool.tile([96, P], f32, tag="pstats")
            nc.tensor.transpose(pstats[:], stats[:, 0:96], id128)

            if KSTAGE >= 3:
                # ---------------- tail ----------------
                psb = lpool.tile([106 if COLTILE else L, 96], f32, tag="psb")
                scratch = lpool.tile([106 if COLTILE else L, 96], f32, tag="scratch")
                Z = lpool.tile([P, 4], f32, tag="Z")
                nc.vector.memset(Z[:], 0.0)
                pacc_f = pacc[:].rearrange("a b c -> a (b c)")
                nc.vector.tensor_copy(psb[0:L], pacc_f[0:L])
                nc.vector.tensor_tensor(
                    scratch[0:L], psb[0:L], W[0:L], op=Alu.mult
                )
                nc.vector.tensor_reduce(
                    Z[0:L, 0:1], scratch[0:L], axis=Axis.X, op=Alu.add
                )
                if COLTILE:
                    nc.vector.tensor_copy(psb[64:64 + L], pacc_f[64:64 + L])
                    nc.vector.tensor_tensor(
                        scratch[64:64 + L], psb[64:64 + L], W[64:64 + L],
                        op=Alu.mult,
                    )
                    nc.vector.tensor_reduce(
                        Z[64:64 + L, 1:2], scratch[64:64 + L], axis=Axis.X,
                        op=Alu.add,
                    )
                # first/last valid labels: pack enc/enc0 pairs into one column,
                # transpose to the free dim, subtract -> labels, one-hot, dot.
                colv = lpool.tile([P, 1], f32, tag="colv")
                nc.vector.memset(colv[:], 0.0)
                # inclusive full-row max = max(exclusive scan end, last partition)
                nc.vector.tensor_tensor(
                    colv[0:NS, 0:1],
                    scanT[:, P - 1 : P],
                    pstatsA[0:NS, P - 1 : P],
                    op=Alu.max,
                )
                nc.vector.tensor_reduce(
                    colv[32:40 + (0 if INTDECODE else 8), 0:1],
                    pstats[32:40 + (0 if INTDECODE else 8), :],
                    axis=Axis.X,
                    op=Alu.min,
                )
                pcv = ppool.tile([1, P], f32, tag="pcv")
                nc.tensor.transpose(pcv[:], colv[:], id128)
                rowT = lpool.tile([1, P], f32, tag="rowT")
                nc.vector.tensor_copy(rowT[:], pcv[:])
                ldF = lpool.tile([1, 8], f32, tag="ldF")
                fdF = lpool.tile([1, 8], f32, tag="fdF")
                if INTDECODE:
                    lfi = lpool.tile([1, 16], i32, tag="lfi")
                    nc.vector.tensor_copy(lfi[0:1, 0:8], rowT[0:1, 0:8])
                    nc.vector.tensor_copy(lfi[0:1, 8:16], rowT[0:1, 32:40])
                    lfm = lpool.tile([1, 16], i32, tag="lfm")
                    nc.vector.tensor_scalar(
                        lfm[:], lfi[:], 63, None, op0=Alu.bitwise_and
                    )
                    nc.vector.tensor_copy(ldF[:], lfm[0:1, 0:8])
                    nc.vector.tensor_copy(fdF[:], lfm[0:1, 8:16])
                else:
                    nc.vector.tensor_tensor(
                        ldF[:], rowT[0:1, 0:8], rowT[0:1, 8:16], op=Alu.subtract
                    )
                    nc.vector.tensor_tensor(
                        fdF[:], rowT[0:1, 32:40], rowT[0:1, 40:48], op=Alu.subtract
                    )
                iota42r = iota48[0:1, 0:42].unsqueeze(1).broadcast_to([1, 8, 42])
                ohf = lpool.tile([1, 8, 42], f32, tag="ohf")
                nc.vector.tensor_tensor(
                    ohf[:],
                    fdF[:].unsqueeze(2).broadcast_to([1, 8, 42]),
                    iota42r,
                    op=Alu.is_equal,
                )
                ohl = lpool.tile([1, 8, 42], f32, tag="ohl")
                nc.vector.tensor_tensor(
                    ohl[:],
                    ldF[:].unsqueeze(2).broadcast_to([1, 8, 42]),
                    iota42r,
                    op=Alu.is_equal,
                )
                sd = lpool.tile([1, 8, 42], f32, tag="sd")
                nc.vector.tensor_tensor(
                    sd[:],
                    ohf[:],
                    tls[0:1, 0:42].unsqueeze(1).broadcast_to([1, 8, 42]),
                    op=Alu.mult,
                )
                nc.vector.tensor_reduce(
                    Z[0:1, 1:2], sd[:], axis=Axis.XY, op=Alu.add
                )
                fd = lpool.tile([1, 8, 42], f32, tag="fd")
                nc.vector.tensor_tensor(
                    fd[:],
                    ohl[:],
                    finrow_sb[0:1, 1:43].unsqueeze(1).broadcast_to([1, 8, 42]),
                    op=Alu.mult,
                )
                nc.vector.tensor_reduce(
                    Z[0:1, 2:3], fd[:], axis=Axis.XY, op=Alu.add
                )
                nc.vector.tensor_reduce(
                    Z[64:72, 3:4], pstats[64:72, :], axis=Axis.X, op=Alu.add
                )
                pout = ppool.tile([4, 1], f32, tag="pout")
                nc.tensor.matmul(pout[:], Z[:], ones, start=True, stop=True)
                outsb = lpool.tile([4, 1], f32, tag="outsb")
                nc.vector.tensor_copy(outsb[:], pout[:])
                nc.sync.dma_start(out_d[:], outsb[:])
            else:
                outsb = lpool.tile([4, 1], f32, tag="outsb")
                if KSTAGE >= 2:
                    psb = lpool.tile([L, 96], f32, tag="psb")
                    nc.vector.tensor_copy(psb[:], pacc[0:L].rearrange("a b c -> a (b c)"))
                    nc.vector.tensor_copy(outsb[:], psb[0:4, 0:1])
                else:
                    nc.vector.tensor_copy(outsb[:], pstats[0:4, 0:1])
                nc.sync.dma_start(out_d[:], outsb[:])

    nc.finalize()
    return nc


def _get_program():
    if "nc" not in _PROGRAM_CACHE:
        _PROGRAM_CACHE["nc"] = build_program()
    return _PROGRAM_CACHE["nc"]


def make_in_maps(log_probs, A_scores, labels, input_lens):
    consts = _host_constants()
    atab = _pack_a_table(np.asarray(A_scores, dtype=np.float32))
    consts["blob32"] = consts["blob32"].copy()
    consts["blob32"][:, F32_TABLE:F32_TABLE + 48] = atab
    in_maps = []
    for c in range(N_CORES):
        sl = slice(c * B_LOC, (c + 1) * B_LOC)
        # pre-permute labels to the on-chip layout [p, r, f], pos = p*64+f,
        # so the device DMA is one contiguous chunk per partition
        lab = np.ascontiguousarray(
            np.asarray(labels[sl], dtype=np.int32)
            .reshape(B_LOC, P, F)
            .transpose(1, 0, 2)
        )
        in_maps.append(
            {
                "lp": np.ascontiguousarray(log_probs[sl], dtype=np.float32),
                "labels": lab,
                **consts,
            }
        )
    return in_maps


def combine_outputs(outs):
    num = 0.0
    tok = 0.0
    for o in outs:
        o = np.asarray(o, dtype=np.float64)
        num += o[0] + o[1] + o[2]
        tok += o[3]
    return np.float32(num / tok)


def kernel(log_probs, A_scores, labels, input_lens):
    nc = _get_program()
    in_maps = make_in_maps(log_probs, A_scores, labels, input_lens)
    res = run_bass_kernel_spmd(nc, in_maps, list(range(N_CORES)))
    return combine_outputs([res.results[c]["out"] for c in range(N_CORES)])


# revision 15
# speedup vs baseline: 1.5715x; 1.0503x over previous
"""Trainium2 Bass kernel for nn_CRFLoss (single-path CRF numerator loss).

Math (matches the reference):
  loss = ( sum_b [ emis_b + lm_b ] ) / num_tokens
  emis_b = sum over valid positions p of log_probs[b, p, labels[b,p]]
  lm_b   = start_lp[s0] + sum_t trans[s_{t-1}, s_t] + fin[s_{T-1}]
           over the sequence of valid labels (s = label - 1)

Device strategy (pure data parallel over batch, 8 rows per core):
  * positions laid out as pos = p*64 + f  (partition p holds 64 consecutive
    positions per row -> fully contiguous DMA of log_probs)
  * log_probs DMA'd in 2-row chunks with SWDGE f32->bf16 cast directly into
    the matmul rhs tiles (no on-chip convert pass, 3 manually rotated bufs)
  * iota/identity constants generated on device (DVE iota + affine_select,
    ACT broadcast-copy); only the emission selector + packed A-table DMA'd
  * one-hot of labels (42 states, bf16) built with DVE is_equal (2x mode),
    two rows per instruction
  * "previous valid label" via encoded running max:
        enc = (pos*64 + label) * valid    (0 = "nothing yet")
    in-partition prefix scan with tensor_tensor_scan(max), cross-partition
    carry via PE transpose + scan + shifted transpose back;
    prev_label = running_max & 63 (int32 cast + bitwise_and)
  * everything accumulates into one PSUM tile through 512 matmuls,
    col-tiled 2x across the PE array (even j -> partitions 0..41,
    odd j -> partitions 64..105):
        psum[c1, 0, c2] += sum_pos onehot[pos,c1] * lp_bf16[pos,c2]
        psum[c1, 1, c3] += sum_pos onehot[pos,c1] * onehot_prev[pos,c3]
    trace of block 0 = emission sum;  block 1 = transition pair counts
  * A_scores log-softmax computed on device (host only re-packs A into a
    padded [43, 48] table = pure layout); softmax + first/last-label work
    interleaved mid-stream so the tail after the last matmul is short
  * final dot products + first/last-label terms assembled into a [128, 4]
    column tile, reduced with a single ones-matmul -> out[4] per core:
        out = [mainA, mainB+start, fin, num_tokens]
  * host: loss = sum_cores(out0+out1+out2) / sum_cores(out3)
"""

import os
import sys

if "/opt/trn_rl_repo" not in sys.path:
    sys.path.insert(0, "/opt/trn_rl_repo")

COLTILE = int(os.environ.get("COLTILE", "1"))
# ACT pexp reads the int32 masked encoding directly (skip f32 convert)
ACTINT = int(os.environ.get("ACTINT", "1"))

import numpy as np

import concourse.bass as bass
import concourse.tile as tile
from concourse import bacc, mybir
from concourse.bass_utils import run_bass_kernel_spmd

# Problem dims (hardcoded per contract)
B, S, C = 64, 8192, 48
L = 42
IGNORE = -100
N_CORES = 8
B_LOC = B // N_CORES  # 8 rows per core
P = 128               # partitions
F = S // P            # 64 positions per partition per row
RPC = 2               # rows per DMA chunk
NCHUNK = B_LOC // RPC
BIG = float(1 << 23)  # sentinel for min-scan; exact in fp32, BIG % 64 == 0

f32 = mybir.dt.float32
bf16 = mybir.dt.bfloat16
i32 = mybir.dt.int32
Alu = mybir.AluOpType
Act = mybir.ActivationFunctionType
Axis = mybir.AxisListType

# f32 const blob layout
F32_W = 0              # [P, 96]   emission selector (rows 0:42 and 64:106)
F32_TABLE = 96         # [43, 48]  padded A-scores table
F32_TOT = 144

_PROGRAM_CACHE = {}


def _host_constants():
    """Data-independent constant tables shipped to each core (1 small blob)."""
    blob32 = np.zeros((P, F32_TOT), dtype=np.float32)
    # emission diag selector: psum[c1, 0, c2] pairs state c1 with class c2;
    # the gold class for state c1 is c1+1.  Replicated at partitions 64..105
    # for the second col-tile's accumulator.
    for c1 in range(L):
        blob32[c1, F32_W + c1 + 1] = 1.0
        blob32[64 + c1, F32_W + c1 + 1] = 1.0
    return {"blob32": blob32}


def _pack_a_table(A_scores):
    """Pure layout: pad A into the [43, 48] per-state table (no math)."""
    t = np.full((P, 48), -1.0e30, dtype=np.float32)
    t[0, 0:L] = A_scores[0:L]
    t[1:L + 1, 0:L + 1] = A_scores[L:].reshape(L, L + 1)
    t[L + 1:, :] = 0.0
    return t


def build_program():
    """Build the per-core Bass/Tile program (SPMD; every core runs this)."""
    nc = bacc.Bacc("TRN2")

    lp_d = nc.declare_dram_parameter("lp", [B_LOC, S, C], f32, isOutput=False)
    lab_d = nc.declare_dram_parameter("labels", [P, B_LOC, F], i32, isOutput=False)
    b32_d = nc.declare_dram_parameter("blob32", [P, F32_TOT], f32, isOutput=False)
    out_d = nc.declare_dram_parameter("out", [4], f32, isOutput=True)

    with tile.TileContext(nc) as tc:
        with (
            tc.tile_pool(name="const", bufs=1) as cpool,
            tc.tile_pool(name="lab", bufs=1) as lpool,
            tc.tile_pool(name="rhs", bufs=1) as rhspool,
            tc.tile_pool(name="ohn", bufs=3) as ohnpool,
            tc.tile_pool(name="prev", bufs=3) as prevpool,
            tc.tile_pool(name="psum", bufs=1, space=bass.MemorySpace.PSUM) as ppool,
        ):
            # ---------------- inputs in ----------------
            # labels alone on the sync/HWDGE ring (fastest path, tiny);
            # const blob leads the gpsimd/SWDGE ring so it completes before
            # the big lp streams that share it (per-ring FIFO ordering).
            lab = lpool.tile([P, B_LOC, F], i32, tag="lab")
            nc.sync.dma_start(lab[:], lab_d[:])
            blob32 = cpool.tile([P, F32_TOT], f32, tag="blob32")
            nc.gpsimd.dma_start(blob32[:], b32_d[:])
            W = blob32[:, F32_W:F32_W + 96]
            table = blob32[0:43, F32_TABLE:F32_TABLE + 48]

            # ---------------- on-device constants ----------------
            iota48 = cpool.tile([P, 48], bf16, tag="iota48")
            nc.gpsimd.iota(
                iota48[:], [[1, 48]], base=1, channel_multiplier=0,
                allow_small_or_imprecise_dtypes=True,
            )
            # iotap[p, f] = (p*64 + f) * 64
            iotap = cpool.tile([P, F], f32, tag="iotap")
            nc.gpsimd.iota(
                iotap[:], [[64, F]], base=0, channel_multiplier=64 * F,
                allow_small_or_imprecise_dtypes=True,
            )
            ones = cpool.tile([P, 1], f32, tag="ones")
            nc.vector.memset(ones[:], 1.0)
            id128 = cpool.tile([P, P], f32, tag="id128")
            nc.gpsimd.affine_select(
                id128[:],
                ones[:, 0:1].broadcast_to([P, P]),
                [[-1, P]],
                compare_op=Alu.is_equal,
                fill=0.0,
                base=0,
                channel_multiplier=1,
            )
            # iota_exp[p, c, f] = c+1 (class-major, contiguous f) on ACT
            iotax = cpool.tile([P, L, F], bf16, tag="iotax")
            nc.scalar.copy(
                iotax[:], iota48[:, 0:L].unsqueeze(2).broadcast_to([P, L, F])
            )

            # ---------------- label prep (DVE) ----------------
            labbf = lpool.tile([P, B_LOC, F], bf16, tag="labbf")
            nc.vector.tensor_copy(labbf[:], lab[:])
            validf = lpool.tile([P, B_LOC, F], f32, tag="validf")
            nc.vector.tensor_scalar(validf[:], lab[:], 0.0, None, op0=Alu.is_gt)
            encb = lpool.tile([P, B_LOC, F], f32, tag="encb")
            iotap_b = iotap[:].unsqueeze(1).broadcast_to([P, B_LOC, F])
            nc.vector.tensor_tensor(encb[:], lab[:], iotap_b, op=Alu.add)
            enc = lpool.tile([P, B_LOC, F], f32, tag="enc")
            nc.vector.tensor_tensor(enc[:], encb[:], validf[:], op=Alu.mult)

            # ---------------- scans ----------------
            # scano[:, r, 0] = 0; scano[:, r, 1+k] = max(enc[:, r, 0..k])
            scano = lpool.tile([P, B_LOC, F + 1], f32, tag="scano")
            nc.vector.memset(scano[:, :, 0:1], 0.0)
            for r in range(B_LOC):
                nc.vector.tensor_tensor_scan(
                    scano[:, r, 1 : F + 1],
                    enc[:, r, :],
                    enc[:, r, :],
                    0.0,
                    op0=Alu.max,
                    op1=Alu.max,
                )
            stats = lpool.tile([P, 96], f32, tag="stats")
            nc.vector.tensor_copy(stats[:, 0:B_LOC], scano[:, :, F])
            # critical-path transpose: per-partition running maxima only
            pstatsA = ppool.tile([8, P], f32, tag="pstatsA")
            nc.tensor.transpose(pstatsA[:], stats[:, 0:8], id128[:])
            # EXCLUSIVE running max of per-partition maxima, per row:
            # scanT[r, p] = max over partitions < p
            scanT = lpool.tile([8, P], f32, tag="scanT")
            nc.vector.memset(scanT[:, 0:1], 0.0)
            nc.vector.tensor_tensor_scan(
                scanT[:, 1:P],
                pstatsA[0:8, 0 : P - 1],
                id128[0:8, 0 : P - 1],
                0.0,
                op0=Alu.max,
                op1=Alu.bypass,
            )
            # back into [128, 8] per-partition carry
            pP = ppool.tile([P, 8], f32, tag="pP")
            nc.tensor.transpose(pP[:], scanT[:], id128[0:8, 0:8])

            # A-scores log-softmax pieces (emitted interleaved with the main
            # loop below so neither DVE nor ACT stalls at the head of a FIFO)
            tmax = lpool.tile([43, 1], f32, tag="tmax")
            x1 = lpool.tile([43, 48], f32, tag="x1")
            ex = lpool.tile([43, 48], f32, tag="ex")
            ssum = lpool.tile([43, 1], f32, tag="ssum")
            lsum = lpool.tile([43, 1], f32, tag="lsum")
            lse = lpool.tile([43, 1], f32, tag="lse")
            tls = lpool.tile([43, 48], f32, tag="tls")
            ptT = ppool.tile([43, 43], f32, tag="ptT")
            finrow = ppool.tile([1, 43], f32, tag="finrow")
            finrow_sb = lpool.tile([1, 43], f32, tag="finrow_sb")

            def softmax_part1():
                nc.vector.tensor_reduce(tmax[:], table, axis=Axis.X, op=Alu.max)
                nc.vector.tensor_scalar(x1[:], table, tmax[:], None, op0=Alu.subtract)
                nc.scalar.activation(ex[:], x1[:], Act.Exp)

            def softmax_part2():
                nc.vector.tensor_reduce(ssum[:], ex[:], axis=Axis.X, op=Alu.add)
                nc.scalar.activation(lsum[:], ssum[:], Act.Ln)
                nc.vector.tensor_tensor(lse[:], tmax[:], lsum[:], op=Alu.add)
                nc.vector.tensor_scalar(tls[:], table, lse[:], None, op0=Alu.subtract)
                # ptT[j, i] = tls[i, j]
                nc.tensor.transpose(ptT[:], tls[0:43, 0:43], id128[0:43, 0:43])
                # W[c1, 48+c3] = trans[state c3 -> state c1] = tls[c3+1, c1]
                nc.vector.tensor_copy(W[0:L, 48:48 + L], ptT[0:L, 1:43])
                if COLTILE:
                    # second col-tile's accumulator needs W at partitions 64+
                    nc.scalar.dma_start(W[64:64 + L, 48:48 + L], W[0:L, 48:48 + L])
                # finrow[0, i] = tls[i, 42]; fin[state c] = finrow[0, c+1]
                nc.tensor.transpose(finrow[:], tls[0:43, 42:43], id128[0:43, 0:43])
                nc.vector.tensor_copy(finrow_sb[:], finrow[:])

            # deferred stats + first/last-label decode, hoisted mid-stream
            encpb = lpool.tile([P, B_LOC, F], f32, tag="encpb")
            encmin = lpool.tile([P, B_LOC, F], f32, tag="encmin")
            pstats = ppool.tile([96, P], f32, tag="pstats")
            Z = lpool.tile([P, 4], f32, tag="Z")
            colv = lpool.tile([P, 1], f32, tag="colv")
            pcv = ppool.tile([1, P], f32, tag="pcv")
            rowT = lpool.tile([1, P], f32, tag="rowT")
            ldF = lpool.tile([1, 8], f32, tag="ldF")
            fdF = lpool.tile([1, 8], f32, tag="fdF")
            lfi = lpool.tile([1, 16], i32, tag="lfi")
            lfm = lpool.tile([1, 16], i32, tag="lfm")
            ohf = lpool.tile([1, 8, 42], f32, tag="ohf")
            ohl = lpool.tile([1, 8, 42], f32, tag="ohl")
            sd = lpool.tile([1, 8, 42], f32, tag="sd")
            fd = lpool.tile([1, 8, 42], f32, tag="fd")

            def stats_block():
                nc.vector.tensor_scalar(encpb[:], enc[:], BIG, None, op0=Alu.add)
                nc.vector.scalar_tensor_tensor(
                    encmin[:], validf[:], -BIG, encpb[:], op0=Alu.mult, op1=Alu.add
                )
                nc.vector.tensor_reduce(
                    stats[:, 32:40], encmin[:], axis=Axis.X, op=Alu.min
                )
                nc.vector.tensor_reduce(
                    stats[:, 64:72], validf[:], axis=Axis.X, op=Alu.add
                )
                nc.tensor.transpose(pstats[:], stats[:, 0:96], id128[:])
                nc.vector.memset(Z[:], 0.0)
                nc.vector.memset(colv[:], 0.0)
                # inclusive full-row max = max(excl scan end, last partition)
                nc.vector.tensor_tensor(
                    colv[0:8, 0:1],
                    scanT[:, P - 1 : P],
                    pstatsA[0:8, P - 1 : P],
                    op=Alu.max,
                )
                nc.vector.tensor_reduce(
                    colv[32:40, 0:1], pstats[32:40, :], axis=Axis.X, op=Alu.min
                )
                nc.tensor.transpose(pcv[:], colv[:], id128[:])
                nc.vector.tensor_copy(rowT[:], pcv[:])
                # first/last labels = (enc encodings) & 63
                nc.vector.tensor_copy(lfi[0:1, 0:8], rowT[0:1, 0:8])
                nc.vector.tensor_copy(lfi[0:1, 8:16], rowT[0:1, 32:40])
                nc.vector.tensor_scalar(
                    lfm[:], lfi[:], 63, None, op0=Alu.bitwise_and
                )
                nc.vector.tensor_copy(ldF[:], lfm[0:1, 0:8])
                nc.vector.tensor_copy(fdF[:], lfm[0:1, 8:16])
                iota42r = iota48[0:1, 0:42].unsqueeze(1).broadcast_to([1, 8, 42])
                nc.vector.tensor_tensor(
                    ohf[:],
                    fdF[:].unsqueeze(2).broadcast_to([1, 8, 42]),
                    iota42r,
                    op=Alu.is_equal,
                )
                nc.vector.tensor_tensor(
                    ohl[:],
                    ldF[:].unsqueeze(2).broadcast_to([1, 8, 42]),
                    iota42r,
                    op=Alu.is_equal,
                )
                nc.vector.tensor_tensor(
                    sd[:],
                    ohf[:],
                    tls[0:1, 0:42].unsqueeze(1).broadcast_to([1, 8, 42]),
                    op=Alu.mult,
                )
                nc.vector.tensor_reduce(
                    Z[0:1, 1:2], sd[:], axis=Axis.XY, op=Alu.add
                )
                nc.vector.tensor_tensor(
                    fd[:],
                    ohl[:],
                    finrow_sb[0:1, 1:43].unsqueeze(1).broadcast_to([1, 8, 42]),
                    op=Alu.mult,
                )
                nc.vector.tensor_reduce(
                    Z[0:1, 2:3], fd[:], axis=Axis.XY, op=Alu.add
                )
                nc.vector.tensor_reduce(
                    Z[64:72, 3:4], pstats[64:72, :], axis=Axis.X, op=Alu.add
                )

            # ---------------- main streaming loop ----------------
            # 3 manually-rotated rhs bufs; block-1 pad cols zeroed once
            rhs_bufs = [
                rhspool.tile([P, RPC, 2, F, C], bf16, name=f"rhs{i}", tag=f"rhs{i}")
                for i in range(3)
            ]
            for rb in rhs_bufs:
                nc.vector.memset(rb[:, :, 1, :, L:C], 0.0)
            # accumulator: even j -> rows 0..41, odd j -> rows 64..105
            pacc = ppool.tile([106 if COLTILE else L, 2, 48], f32, tag="pacc")
            for k in range(NCHUNK):
                rhs_t = rhs_bufs[k % 3]
                # SWDGE casts f32->bf16 in the DMA datapath
                nc.gpsimd.dma_start(
                    rhs_t[:, :, 0],
                    lp_d[k * RPC : (k + 1) * RPC].rearrange(
                        "r (p f) c -> p r f c", p=P
                    ),
                )
                r0 = k * RPC
                # class-major one-hot vs expanded iota const (2x, both rows)
                ohn = ohnpool.tile([P, RPC, L, F], bf16, tag="ohn")
                nc.vector.tensor_tensor(
                    ohn[:],
                    labbf[:, r0:r0 + RPC, :].unsqueeze(2).broadcast_to(
                        [P, RPC, L, F]
                    ),
                    iotax[:].unsqueeze(1).broadcast_to([P, RPC, L, F]),
                    op=Alu.is_equal,
                )
                # prev_enc = max(in-partition exclusive scan, cross-part carry)
                prevb = prevpool.tile([P, RPC, F], f32, tag="prevb")
                nc.vector.tensor_tensor(
                    prevb[:],
                    scano[:, r0:r0 + RPC, 0:F],
                    pP[:, r0:r0 + RPC].unsqueeze(2).broadcast_to([P, RPC, F]),
                    op=Alu.max,
                )
                # prev label = enc & 63 (positions encoded as pos*64)
                previ = prevpool.tile([P, RPC, F], i32, tag="previ")
                nc.vector.tensor_copy(previ[:], prevb[:])
                prevm = prevpool.tile([P, RPC, F], i32, tag="prevm")
                nc.vector.tensor_scalar(
                    prevm[:], previ[:], 63, None, op0=Alu.bitwise_and
                )
                if ACTINT:
                    pexp_src = prevm
                else:
                    prevl = prevpool.tile([P, RPC, F], f32, tag="prevl")
                    nc.vector.tensor_copy(prevl[:], prevm[:])
                    pexp_src = prevl
                # ACT expands prev labels so the is_equal runs 2x packed
                pexp = prevpool.tile([P, RPC, F, C], bf16, tag="pexp")
                nc.scalar.copy(
                    pexp[:], pexp_src[:].unsqueeze(3).broadcast_to([P, RPC, F, C])
                )
                nc.vector.tensor_tensor(
                    rhs_t[:, :, 1, :, 0:L],
                    pexp[:, :, :, 0:L],
                    iota48[:, 0:L].unsqueeze(1).unsqueeze(1).broadcast_to(
                        [P, RPC, F, L]
                    ),
                    op=Alu.is_equal,
                )
                for rr in range(RPC):
                    r = r0 + rr
                    for j in range(F):
                        if COLTILE:
                            tgt = pacc[0:L] if j % 2 == 0 else pacc[64:64 + L]
                            first = r == 0 and j < 2
                            last = r == B_LOC - 1 and j >= F - 2
                        else:
                            tgt = pacc[0:L]
                            first = r == 0 and j == 0
                            last = r == B_LOC - 1 and j == F - 1
                        nc.tensor.matmul(
                            tgt,
                            ohn[:, rr, :, j],
                            rhs_t[:, rr, :, j, :],
                            start=first,
                            stop=last,
                            skip_group_check=True,
                        )
                if k == 0:
                    softmax_part1()
                    softmax_part2()
                elif k == 1:
                    stats_block()

            # ---------------- tail ----------------
            NP = 106 if COLTILE else L
            psb = lpool.tile([NP, 96], f32, tag="psb")
            if COLTILE:
                # zero the unused middle partitions (32-aligned start; rows
                # 32:42 get overwritten by the copy below)
                nc.vector.memset(psb[32:64, :], 0.0)
            scratch = lpool.tile([NP, 96], f32, tag="scratch")
            pacc_f = pacc[:].rearrange("a b c -> a (b c)")
            nc.vector.tensor_copy(psb[0:L], pacc_f[0:L])
            if COLTILE:
                nc.vector.tensor_copy(psb[64:64 + L], pacc_f[64:64 + L])
            nc.vector.tensor_tensor(
                scratch[0:NP], psb[0:NP], W[0:NP], op=Alu.mult
            )
            nc.vector.tensor_reduce(
                Z[0:NP, 0:1], scratch[0:NP], axis=Axis.X, op=Alu.add
            )
            pout = ppool.tile([4, 1], f32, tag="pout")
            nc.tensor.matmul(pout[:], Z[:], ones[:], start=True, stop=True)
            outsb = lpool.tile([4, 1], f32, tag="outsb")
            nc.vector.tensor_copy(outsb[:], pout[:])
            nc.sync.dma_start(out_d[:], outsb[:])

    nc.finalize()
    return nc


def _get_program():
    if "nc" not in _PROGRAM_CACHE:
        _PROGRAM_CACHE["nc"] = build_program()
    return _PROGRAM_CACHE["nc"]


def make_in_maps(log_probs, A_scores, labels, input_lens):
    consts = _host_constants()
    atab = _pack_a_table(np.asarray(A_scores, dtype=np.float32))
    blob32 = consts["blob32"].copy()
    blob32[:, F32_TABLE:F32_TABLE + 48] = atab
    in_maps = []
    for c in range(N_CORES):
        sl = slice(c * B_LOC, (c + 1) * B_LOC)
        # pre-permute labels to the on-chip layout [p, r, f], pos = p*64+f,
        # so the device DMA is one contiguous chunk per partition
        lab = np.ascontiguousarray(
            np.asarray(labels[sl], dtype=np.int32)
            .reshape(B_LOC, P, F)
            .transpose(1, 0, 2)
        )
        in_maps.append(
            {
                "lp": np.ascontiguousarray(log_probs[sl], dtype=np.float32),
                "labels": lab,
                "blob32": blob32,
            }
        )
    return in_maps


def combine_outputs(outs):
    num = 0.0
    tok = 0.0
    for o in outs:
        o = np.asarray(o, dtype=np.float64)
        num += o[0] + o[1] + o[2]
        tok += o[3]
    return np.float32(num / tok)


def kernel(log_probs, A_scores, labels, input_lens):
    nc = _get_program()
    in_maps = make_in_maps(log_probs, A_scores, labels, input_lens)
    res = run_bass_kernel_spmd(nc, in_maps, list(range(N_CORES)))
    return combine_outputs([res.results[c]["out"] for c in range(N_CORES)])
